# revision 2
# baseline (speedup 1.0000x reference)
"""MinimumErrorRateLoss on 8 Trainium2 NeuronCores — row-scan DP kernel.

The loss is dominated by B = N*M = 4096 independent Levenshtein edit
distances (ref length R=256 vs hyp length H=288). Pairs are sharded 512 per
core, laid out as 128 SBUF partitions x 4 free-dim segments.

Instead of an anti-diagonal wavefront (544 diagonals x 4 ops, ~2200
instructions), the DP runs row-wise over the H hyp steps in "p-space":
with p_t[i] = D[i][t] - i - t, the three-way Levenshtein recurrence
collapses into a single prefix-min, so each hyp step is only THREE
VectorEngine ops:

    eq[i]  = (ref[i-1] == hyp[t])            # tensor_tensor is_equal
    s[i]   = (p[i-1] - 1) - eq[i]            # scalar_tensor_tensor
    p'[i]  = min(p[i], s[i], p'[i-1])        # tensor_tensor_scan (hw scan)

The scan runs flat across the 4 segments of each partition; descending
per-segment offsets (+3072/+2048/+1024/+0, step > 2*min(R,H)+2) make the
scan state that leaks across a segment boundary strictly larger than any
value in the next segment, so the flat scan is exact. The H steps run in a
hardware Fori loop (4 steps per iteration, eq pipelined one step ahead and
placed after the scan so it overlaps the scan's write flush; one drain per
step orders the scan's output for its consumer). The NEFF is ~30
instructions. dist = p_H[R] - offset + R + H; softmax/mean on host.
"""

import numpy as np

N, M, R, H = 128, 32, 256, 288
NCORES = 8
P = 128          # SBUF partitions
SEG = 4          # segments per partition -> 512 pairs per core
BPC = P * SEG
UNROLL = 4       # DP steps per hardware-loop iteration
SLOTS = R + 2    # p/s slots per segment: i=0..R plus one pad
BIG = 60000.0
OFFSTEP = 1024.0  # > 2*min(R,H)+2 so cross-segment scan leakage is inert

_CACHE = {}


def _build_program(r, h, reps=1, unroll=UNROLL):
    from contextlib import ExitStack

    import concourse.bass as bass
    import concourse.mybir as mybir

    slots = r + 2
    in_stride = slots + h + unroll
    nc = bass.Bass(
        "TRN2", target_bir_lowering=False, debug=False,
        detect_race_conditions=False,
    )
    dt = mybir.dt.float32
    inp = nc.dram_tensor(
        "inp", [P, SEG, in_stride], dt, kind="ExternalInput"
    ).ap()
    dist_out = nc.dram_tensor("dist", [P, SEG, 1], dt, kind="ExternalOutput").ap()
    AOT = mybir.AluOpType

    with ExitStack() as ctx:
        inpt = ctx.enter_context(nc.sbuf_tensor("inpt", [P, SEG, in_stride], dt))
        p0 = ctx.enter_context(nc.sbuf_tensor("p0", [P, SEG, slots], dt))
        p1 = ctx.enter_context(nc.sbuf_tensor("p1", [P, SEG, slots], dt))
        sb = ctx.enter_context(nc.sbuf_tensor("sb", [P, SEG, slots], dt))
        sb2 = ctx.enter_context(nc.sbuf_tensor("sb2", [P, SEG, slots], dt))
        eqs = [
            ctx.enter_context(nc.sbuf_tensor(f"eq{k}", [P, SEG, r], dt))
            for k in range(unroll)
        ]
        outt = ctx.enter_context(nc.sbuf_tensor("outt", [P, SEG, 1], dt))
        dma_sem = ctx.enter_context(nc.semaphore("dma_sem"))
        vdone = ctx.enter_context(nc.semaphore("vdone"))
        block = ctx.enter_context(nc.Block())

        reft = inpt[:, :, 1 : r + 1]          # ref symbol a[i-1] at slot i
        hyp0 = inpt[:, :, slots : slots + h]  # hyp[t] at slot slots+t
        hypv = [
            inpt[:, :, slots + k : slots + h + k] for k in range(unroll + 1)
        ]  # hypv[k][t] = hyp[t+k]; tail reads land in zero pad slots

        def flat(t):  # 2D view for the scan: free dims are contiguous
            return t[:].rearrange("p s k -> p (s k)")

        # The NEFF may be executed repeatedly on one load: clear semaphores
        # at the END of each run so every execution starts from zero.
        @block.gpsimd
        def _(gpsimd):
            gpsimd.wait_ge(dma_sem, 32)
            gpsimd.sem_clear(dma_sem)
            gpsimd.sem_clear(vdone)

        @block.sync
        def _(sync):
            sync.dma_start(out=inpt[:], in_=inp).then_inc(dma_sem, 16)
            sync.wait_ge(vdone, 1)
            sync.dma_start(out=dist_out, in_=outt[:]).then_inc(dma_sem, 16)

        @block.vector
        def _(vector):
            import concourse.bass as bass_mod

            vector.wait_ge(dma_sem, 16)
            assert h % unroll == 0 and unroll % 2 == 0
            with vector.Fori(0, reps) as _rep:
                vector.drain()
                vector.memset(p0[:], BIG)
                for g in range(SEG):
                    # p_0[i] = 0 + segment offset; pad slot keeps BIG
                    vector.memset(p0[:, g, 0 : r + 1], (SEG - 1 - g) * OFFSTEP)
                vector.memset(sb[:], BIG)   # slot 0 + pad stay BIG forever
                vector.memset(sb2[:], BIG)
                # prologue: eq for t=0 (static slice)
                h0 = hyp0[:, :, 0:1].broadcast_to((P, SEG, r))
                vector.tensor_tensor(out=eqs[0][:], in0=reft, in1=h0,
                                     op=AOT.is_equal)
                vector.drain()
                with vector.Fori(0, h, unroll) as t:
                    pa, pb = p0, p1
                    for k in range(unroll):
                        sbuf_k = sb if k % 2 == 0 else sb2
                        # s[i] = (p[i-1] - 1) - eq[i], slots 1..r
                        vector.scalar_tensor_tensor(
                            out=sbuf_k[:, :, 1 : r + 1],
                            in0=pa[:, :, 0:r], scalar=-1.0,
                            in1=eqs[k][:], op0=AOT.add, op1=AOT.subtract,
                        )
                        # p'[i] = min(p[i], s[i], p'[i-1]); the scan's reads
                        # lag its writes, so reading s right after the STT
                        # is safe (sequential scan).
                        vector.tensor_tensor_scan(
                            out=flat(pb), data0=flat(pa), data1=flat(sbuf_k),
                            initial=BIG, op0=AOT.min, op1=AOT.min,
                        )
                        # eq for step t+k+1, executed while the scan's
                        # writes flush; consumed after the drain.
                        hcol = hypv[k + 1][:, :, bass_mod.ds(t, 1)] \
                            .broadcast_to((P, SEG, r))
                        vector.tensor_tensor(
                            out=eqs[(k + 1) % unroll][:], in0=reft,
                            in1=hcol, op=AOT.is_equal,
                        )
                        # consumers of a scan's output need a drain; plain
                        # op-to-op seams do not (measured).
                        vector.drain()
                        pa, pb = pb, pa
            vector.tensor_copy(
                out=outt[:], in_=p0[:, :, r : r + 1]
            ).then_inc(vdone, 1)
    return nc


def _get_program(r=R, h=H, reps=1, unroll=UNROLL):
    key = (r, h, reps, unroll)
    if key not in _CACHE:
        _CACHE[key] = _build_program(r, h, reps, unroll=unroll)
    return _CACHE[key]


def _make_in_maps(ref_pair_f32, hyp_pair_f32, r=R, h=H, unroll=UNROLL):
    """ref_pair_f32: (B, r) float32; hyp_pair_f32: (B, h). b_local = g*128+p."""
    slots = r + 2
    in_stride = slots + h + unroll
    in_maps = []
    for c in range(NCORES):
        lo = c * BPC
        ra = ref_pair_f32[lo : lo + BPC].reshape(SEG, P, r).transpose(1, 0, 2)
        ha = hyp_pair_f32[lo : lo + BPC].reshape(SEG, P, h).transpose(1, 0, 2)
        arr = np.zeros((P, SEG, in_stride), np.float32)
        arr[:, :, 1 : r + 1] = ra
        arr[:, :, slots : slots + h] = ha
        in_maps.append({"inp": arr})
    return in_maps


def _gather_dist(results, r=R, h=H):
    dist = np.empty(NCORES * BPC, np.float32)
    offs = np.array([(SEG - 1 - g) * OFFSTEP for g in range(SEG)], np.float32)
    for c in range(NCORES):
        d = np.asarray(results[c]["dist"]).reshape(P, SEG)  # [p, g]
        d = d - offs[None, :] + np.float32(r + h)
        dist[c * BPC : (c + 1) * BPC] = d.T.reshape(BPC)
    return dist


def run_device_dp(ref_pair_f32, hyp_pair_f32, r=R, h=H, trace=False):
    """Run the sharded edit-distance DP on the 8 NeuronCores."""
    from concourse.bass_utils import run_bass_kernel_spmd

    nc = _get_program(r, h)
    in_maps = _make_in_maps(ref_pair_f32, hyp_pair_f32, r, h)
    res = run_bass_kernel_spmd(nc, in_maps, list(range(NCORES)), trace=trace)
    return _gather_dist(res.results, r, h), res


def kernel(log_probs, ref, hyp):
    """Full-input entry point. log_probs (128,32) f32, ref (256,128) int,
    hyp (288,128,32) int -> scalar float32 loss."""
    B = N * M
    refT = np.ascontiguousarray(np.asarray(ref).astype(np.float32).T)
    hypT = np.ascontiguousarray(
        np.asarray(hyp).astype(np.float32).transpose(1, 2, 0)
    )
    bidx = np.arange(B)
    ref_pair = refT[bidx // M]
    hyp_pair = hypT[bidx // M, bidx % M]

    dist, _ = run_device_dp(ref_pair, hyp_pair)

    er = (dist / np.float32(R)).reshape(N, M)
    er = er - er.mean(axis=1, keepdims=True, dtype=np.float32)
    lp = np.asarray(log_probs).astype(np.float32)
    ex = np.exp(lp - lp.max(axis=1, keepdims=True))
    sm = ex / ex.sum(axis=1, keepdims=True, dtype=np.float32)
    return np.asarray((er * sm).mean(dtype=np.float32), dtype=np.float32)
